# revision 26
# baseline (speedup 1.0000x reference)
"""Gated multi-head attention block on 8 Trainium2 NeuronCores.

Computes, for B=128, S=200, D=256, H=8:
    e_td = (concat(E_t, E_d) @ W_td)            -> per-(b, dh) gate
    Q/K/V = (E_h @ W_{Q,K,V}) * e_td            -> [B, S, H, D]
    out   = softmax(Q K^T / sqrt(D)) V  @ W_out -> [B, S, D]

Strategy: data-parallel over batch (16 per core), weights replicated.
All matmuls run as float32r (full fp32 operands, 1 cycle/row at N>=256),
with the token dimension padded 200->256. Scores are computed transposed
([k, q]) so the softmax sum over k is a partition-axis reduction done with
GPSIMD partition_all_reduce; exp runs on ACT with the 1/sqrt(D) scale fused;
all gating is fused into the PSUM->SBUF copies as per-partition scales.
"""

import numpy as np
from contextlib import ExitStack

import concourse.bass as bass
import concourse.mybir as mybir
import concourse.tile as tile
from concourse import bacc, bass_utils, library_config
from concourse.bass_isa import ReduceOp

F32 = mybir.dt.float32
F32R = mybir.dt.float32r
AF = mybir.ActivationFunctionType

P = 128          # partitions
B, S, D, H = 128, 200, 256, 8
NCORES = 8
NB = B // NCORES  # batches per core
SP = 256          # padded token dim (>=256 keeps float32r at 1 cyc/row)
KREM = S - P      # 72: second k/q block height
DH = D * H        # 2048
NG = DH // P      # 16 dh groups of 128

_CACHED = {}


def _emit(tc, d, repeat=1):
    nc = tc.nc
    ctx = ExitStack()
    with ctx:
        # FP32R matmul operands must be written by a compute op with
        # float32r output dtype (the "rounded to FP32r" birverifier rule),
        # so every DMA'd matmul input goes through a one-time rounding copy.
        wpool = ctx.enter_context(tc.tile_pool(name="w", bufs=1))
        wq_sb = wpool.tile([P, 2, DH], F32R, name="wq_sb", tag="wq")
        wk_sb = wpool.tile([P, 2, DH], F32R, name="wk_sb", tag="wk")
        wv_sb = wpool.tile([P, 2, DH], F32R, name="wv_sb", tag="wv")
        wout_sb = wpool.tile([P, NG, D], F32R, name="wout_sb", tag="wout")
        gates = wpool.tile([P, NG, NB], F32, name="gates", tag="gates")
        etdt_sb = wpool.tile([P, 4, NB], F32R, name="etdt_sb", tag="etdt")

        # partition_all_reduce lives in the attnmlp GPSIMD ucode library;
        # load it once (a reload costs ~6us, so GPSIMD runs ONLY attnmlp ops)
        nc.gpsimd.load_library(library_config.attnmlp)

        psA = ctx.enter_context(tc.tile_pool(name="psA", bufs=6, space="PSUM"))
        psO = ctx.enter_context(tc.tile_pool(name="psO", bufs=2, space="PSUM"))

        with tc.tile_pool(name="wstage", bufs=2) as wst:
            for wdst, wsrc in ((wq_sb, d["wq"]), (wk_sb, d["wk"]),
                               (wv_sb, d["wv"])):
                stg = wst.tile([P, 2, DH], F32, name=f"stg_{wsrc.name}",
                               tag="wstage")
                nc.sync.dma_start(stg[:], wsrc)
                for half in range(2):
                    nc.vector.tensor_copy(wdst[:, half, :], stg[:, half, :])
            stg = wst.tile([P, NG, D], F32, name="stg_wout", tag="wstage")
            nc.sync.dma_start(stg[:], d["wout"])
            for half in range(2):
                nc.vector.tensor_copy(
                    wout_sb[:, half * 8:(half + 1) * 8, :],
                    stg[:, half * 8:(half + 1) * 8, :],
                )
            stge = wst.tile([P, 4, NB], F32, name="stg_etdt", tag="estage")
            nc.gpsimd.dma_start(stge[:], d["etdt"])
            nc.vector.tensor_copy(etdt_sb[:], stge[:])

            # --- e_td phase: gates[dh, b] = (concat(E_t, E_d) @ W_td)^T ---
            # W_td streamed in 512-wide output chunks to keep SBUF low.
            for nc4 in range(4):
                wtd_s = wst.tile([P, 4, 512], F32, name=f"wtds_{nc4}",
                                 tag="wtds")
                nc.sync.dma_start(wtd_s[:], d["wtd"][:, :, nc4 * 512:(nc4 + 1) * 512])
                wtd_c = wst.tile([P, 4, 512], F32R, name=f"wtd_{nc4}",
                                 tag="wtdc")
                nc.vector.tensor_copy(wtd_c[:], wtd_s[:])
                for gi in range(4):
                    g = nc4 * 4 + gi
                    pt = psA.tile([P, NB], F32, name=f"ps_td{g}", tag="psA")
                    for kt in range(4):
                        nc.tensor.matmul(
                            pt[:],
                            wtd_c[:, kt, gi * P:(gi + 1) * P],
                            etdt_sb[:, kt, :],
                            start=(kt == 0),
                            stop=(kt == 3),
                        )
                    nc.vector.tensor_copy(gates[:, g, :], pt[:])

        # --- persistent per-b tensors ---
        big = ctx.enter_context(tc.tile_pool(name="big", bufs=1))
        qt = big.tile([P, NG, SP], F32R, name="qt", tag="qt")      # Q^T gated
        ktr = big.tile([P, NG, SP], F32R, name="ktr", tag="ktr")   # K^T gated
        vsb = big.tile([P, 2, DH], F32R, name="vsb", tag="vsb")    # V token-major
        expt = big.tile([P, NG, SP], F32R, name="expt", tag="expt")  # exp(scores^T)
        att = big.tile([P, NG, SP], F32R, name="att", tag="att")   # attn^T gated

        # One-time zeroing of pad regions (they stay zero forever):
        #  - token-pad cols 200:256 of qt/ktr/expt/att (ops below only touch
        #    [:200]; the fp32r matmuls still stream all 256 cols)
        #  - k-pad rows of odd exp groups (exp only writes [:72]; engine APs
        #    must start at partition 0/32/64/96, so start at 64)
        # (memset can't write float32r, so copy from a zeroed f32 tile)
        zpad = big.tile([P, SP], F32, name="zpad", tag="zpad")
        nc.vector.memset(zpad[:], 0.0)
        for g in range(NG):
            nc.vector.tensor_copy(qt[:, g, S:SP], zpad[:, S:SP])
            nc.vector.tensor_copy(ktr[:, g, S:SP], zpad[:, S:SP])
            nc.vector.tensor_copy(att[:, g, S:SP], zpad[:, S:SP])

        ehp = ctx.enter_context(tc.tile_pool(name="ehp", bufs=2))
        scr = ctx.enter_context(tc.tile_pool(name="scr", bufs=3))

        for rep in range(repeat):
          for b in range(NB):
            # casting SWDGE DMA (gpsimd) satisfies the fp32r rounding rule
            ehb = ehp.tile([P, 2, SP], F32R, name=f"ehb{rep}_{b}", tag="ehb")
            nc.gpsimd.dma_start(ehb[:], d["eht"][:, :, b * SP:(b + 1) * SP])

            # Q^T / K^T projections, gate applied during PSUM->SBUF copy
            for g in range(NG):
                pq = psA.tile([P, SP], F32, name=f"pq{rep}_{b}_{g}", tag="psA")
                for dt in range(2):
                    nc.tensor.matmul(
                        pq[:],
                        wq_sb[:, dt, g * P:(g + 1) * P],
                        ehb[:, dt, :],
                        start=(dt == 0), stop=(dt == 1),
                    )
                nc.scalar.mul(qt[:, g, :S], pq[:, :S], gates[:, g, b:b + 1])
                pk = psA.tile([P, SP], F32, name=f"pk{rep}_{b}_{g}", tag="psA")
                for dt in range(2):
                    nc.tensor.matmul(
                        pk[:],
                        wk_sb[:, dt, g * P:(g + 1) * P],
                        ehb[:, dt, :],
                        start=(dt == 0), stop=(dt == 1),
                    )
                # 12 K copies on DVE, 4 on ACT (engine load balance)
                if g % 4 == 3:
                    nc.scalar.mul(ktr[:, g, :S], pk[:, :S], gates[:, g, b:b + 1])
                else:
                    nc.vector.tensor_scalar_mul(
                        ktr[:, g, :S], pk[:, :S], gates[:, g, b:b + 1]
                    )

            # V projection, ungated (its gate is folded into the attn^T copy)
            for kb in range(2):
                for nc4 in range(4):
                    pv = psA.tile([P, 512], F32, name=f"pv{rep}_{b}_{kb}_{nc4}", tag="psA")
                    for dt in range(2):
                        nc.tensor.matmul(
                            pv[:],
                            ehb[:, dt, kb * P:(kb + 1) * P],
                            wv_sb[:, dt, nc4 * 512:(nc4 + 1) * 512],
                            start=(dt == 0), stop=(dt == 1),
                        )
                    nc.vector.tensor_copy(vsb[:, kb, nc4 * 512:(nc4 + 1) * 512], pv[:])

            # Phase 1: scores^T + exp for all heads.  Both k-blocks of a
            # head land in one PSUM bank so a single ACT exp covers them.
            # The k-pad rows of the second block hold scores==0 -> exp==1;
            # their denominator contribution (exactly 56) is subtracted in
            # phase 2, and attn^T is immune (V's pad token rows are zero).
            for h in range(H):
                ps2 = psA.tile([P, 2 * SP], F32, name=f"ps{rep}_{b}_{h}", tag="psA")
                for kb in range(2):
                    for dt in range(2):
                        nc.tensor.matmul(
                            ps2[:, kb * SP:(kb + 1) * SP],
                            ktr[:, 2 * h + dt, kb * P:(kb + 1) * P],
                            qt[:, 2 * h + dt, :],
                            start=(dt == 0), stop=(dt == 1),
                        )
                nc.scalar.activation(
                    expt[:, 2 * h:2 * h + 2, :],
                    ps2.rearrange("p (a c) -> p a c", a=2),
                    AF.Exp, scale=0.0625,
                )
            # Phase 2: softmax denominators.  GPSIMD: partition allreduce
            # (the only op its loaded ucode library has); DVE: pair-add,
            # reciprocal, and the in-place normalize.
            for h in range(H):
                # full contiguous 512-wide AP: strided in/out views confuse
                # the allreduce (and pad cols are zero, so they sum to zero)
                ssum = scr.tile([P, 2, SP], F32, name=f"ssum{rep}_{b}_{h}",
                                tag="ssum", bufs=3)
                nc.gpsimd.partition_all_reduce(
                    ssum[:], expt[:, 2 * h:2 * h + 2, :], P, ReduceOp.add
                )
                den = scr.tile([P, S], F32, name=f"den{rep}_{b}_{h}", tag="den")
                nc.vector.scalar_tensor_tensor(
                    den[:], ssum[:, 0, :S], -56.0, ssum[:, 1, :S],
                    op0=mybir.AluOpType.add, op1=mybir.AluOpType.add,
                )
                rden = scr.tile([P, S], F32, name=f"rden{rep}_{b}_{h}",
                                tag="rden", bufs=3)
                nc.vector.reciprocal_approx_fast(rden[:], den[:])
                nc.vector.tensor_mul(
                    expt[:, 2 * h:2 * h + 2, :S],
                    expt[:, 2 * h:2 * h + 2, :S],
                    rden[:, None, :].to_broadcast([P, 2, S]),
                )
            # Phase 3: attn^T + gated copy-out
            for h in range(H):
                for ds in range(2):
                    g = 2 * h + ds
                    pa = psA.tile([P, SP], F32, name=f"pa{rep}_{b}_{h}_{ds}", tag="psA")
                    for kb in range(2):
                        nc.tensor.matmul(
                            pa[:],
                            vsb[:, kb, g * P:(g + 1) * P],
                            expt[:, 2 * h + kb, :],
                            start=(kb == 0), stop=(kb == 1),
                        )
                    nc.scalar.mul(att[:, g, :S], pa[:, :S], gates[:, g, b:b + 1])

            # output projection: out[q, :] = sum_g att[:, g, q]^T @ wout[g]
            for qb in range(2):
                po = psO.tile([P, D], F32, name=f"po{rep}_{b}_{qb}", tag="psO")
                for g in range(NG):
                    nc.tensor.matmul(
                        po[:],
                        att[:, g, qb * P:(qb + 1) * P],
                        wout_sb[:, g, :],
                        start=(g == 0), stop=(g == NG - 1),
                    )
                rows = P if qb == 0 else KREM
                osb = scr.tile([P, D], F32, name=f"osb{rep}_{b}_{qb}", tag="osb")
                nc.scalar.copy(osb[:rows, :], po[:rows, :])
                nc.sync.dma_start(d["out"][b, qb * P:qb * P + rows, :], osb[:rows, :])


def build_nc(repeat=1):
    # Bacc (not raw Bass): its compile() pass legalizes multi-sem waits into
    # sequencer waits (HW instructions encode only one wait command) and
    # lowers the extended GPSIMD ISA instructions.
    nc = bacc.Bacc("TRN2", target_bir_lowering=False, debug=False,
                   num_devices=NCORES)
    d = {
        "eht": nc.dram_tensor("eht", [P, 2, NB * SP], F32, kind="ExternalInput").ap(),
        "etdt": nc.dram_tensor("etdt", [P, 4, NB], F32, kind="ExternalInput").ap(),
        "wq": nc.dram_tensor("wq", [P, 2, DH], F32, kind="ExternalInput").ap(),
        "wk": nc.dram_tensor("wk", [P, 2, DH], F32, kind="ExternalInput").ap(),
        "wv": nc.dram_tensor("wv", [P, 2, DH], F32, kind="ExternalInput").ap(),
        "wtd": nc.dram_tensor("wtd", [P, 4, DH], F32, kind="ExternalInput").ap(),
        "wout": nc.dram_tensor("wout", [P, NG, D], F32, kind="ExternalInput").ap(),
        "out": nc.dram_tensor("out", [NB, S, D], F32, kind="ExternalOutput").ap(),
    }
    with tile.TileContext(nc) as tc:
        _emit(tc, d, repeat=repeat)
    nc.compile()
    return nc


def _split_k(w, kt):
    # [K, N] -> [128, kt, N] with row k = t*128 + p
    K, N = w.shape
    return np.ascontiguousarray(
        w.reshape(kt, P, N).transpose(1, 0, 2)
    ).astype(np.float32)


def make_in_maps(E_h, E_t, E_d, W_Q, W_K, W_V, W_td, W_out):
    wq = _split_k(np.asarray(W_Q, np.float32), 2)
    wk = _split_k(np.asarray(W_K, np.float32), 2)
    wv = _split_k(np.asarray(W_V, np.float32), 2)
    wtd = _split_k(np.asarray(W_td, np.float32), 4)
    wout = _split_k(np.asarray(W_out, np.float32), NG)
    E_h = np.asarray(E_h, np.float32)
    E_t = np.asarray(E_t, np.float32)
    E_d = np.asarray(E_d, np.float32)

    in_maps = []
    for c in range(NCORES):
        sl = slice(c * NB, (c + 1) * NB)
        # E_h^T padded: [256(d), NB, SP] with zeros in token cols 200:256
        x = np.zeros((2 * P, NB, SP), np.float32)
        x[:, :, :S] = E_h[sl].transpose(2, 0, 1)
        eht = np.ascontiguousarray(
            x.reshape(2, P, NB * SP).transpose(1, 0, 2)
        )
        cat = np.concatenate([E_t[sl], E_d[sl]], axis=1)  # [NB, 512]
        etdt = _split_k(np.ascontiguousarray(cat.T), 4)   # [128, 4, NB]
        in_maps.append({
            "eht": eht, "etdt": etdt, "wq": wq, "wk": wk, "wv": wv,
            "wtd": wtd, "wout": wout,
        })
    return in_maps


def get_nc():
    if "nc" not in _CACHED:
        _CACHED["nc"] = build_nc()
    return _CACHED["nc"]


def run(inputs, **kw):
    """Build + run on 8 cores; returns (full_output, BassKernelResults)."""
    nc = get_nc()
    in_maps = make_in_maps(**inputs)
    res = bass_utils.run_bass_kernel_spmd(
        nc, in_maps, core_ids=list(range(NCORES)), **kw
    )
    out = np.concatenate([r["out"] for r in res.results], axis=0)
    return np.ascontiguousarray(out, dtype=np.float32), res


def kernel(**inputs) -> np.ndarray:
    out, _ = run(inputs)
    return out
